# revision 14
# baseline (speedup 1.0000x reference)
"""Causal self-attention (RoPE) Trainium2 Bass kernel, v2 (bf16).

Sharding: 8 cores = 4 batches x 2 head-groups. Core c handles batch c//2 and
heads (c%2)*8 .. (c%2)*8+7. Each core computes its QKV projection slice, RoPE,
causal flash-style attention in transposed layout, and a partial output
projection; the host sums the two partial projections per batch.

v2 changes vs baseline:
- all matmuls bf16 (fp32r ran in fp32-HIGH mode at half PE rate and kept the
  HAM clock gate cold for ~2/3 of the kernel)
- attention inner loop software-pipelined: scores for tile k+1 issue on PE
  before the AV matmuls of tile k, so exp (ACT) and mask (DVE) overlap PE
- reciprocal_approx_fast for softmax denominators (plain RECIPROCAL was 3.3us
  per call, 107us total)
- causal mask multiply only on the 128-wide diagonal triangle
- single pass over x feeds both the V and QK projections
- projection / attention / output-projection emission interleaved per
  512-query chunk so the PE queue always has ready work behind stalled ops
"""

import math
import numpy as np
from contextlib import ExitStack

import ml_dtypes

import concourse.bass as bass
import concourse.tile as tile
from concourse import bacc, mybir
from concourse.bass_utils import run_bass_kernel_spmd

F32 = mybir.dt.float32
BF16 = mybir.dt.bfloat16
EXPF = mybir.ActivationFunctionType.Exp
MULT = mybir.AluOpType.mult
ADD = mybir.AluOpType.add

B, T, C, H, D = 4, 2048, 1024, 16, 64
HL = 8            # local heads per core
NP = HL // 2      # head pairs per core
KT = C // 128     # contraction tiles for projections
TT = T // 128     # 128-row tiles of T
QC = T // 512     # 512-col chunks of T
SCALE = 1.0 / math.sqrt(D)

_CACHE = {}


def _build_nc():
    nc = bacc.Bacc("TRN2", debug=False, num_devices=8)

    xT_d = nc.dram_tensor("xT", [128, KT, T], BF16, kind="ExternalInput").ap()
    wq_d = nc.dram_tensor("wq", [128, NP, KT, 128], BF16, kind="ExternalInput").ap()
    wk_d = nc.dram_tensor("wk", [128, NP, KT, 128], BF16, kind="ExternalInput").ap()
    wv_d = nc.dram_tensor("wv", [128, KT, 512], BF16, kind="ExternalInput").ap()
    wo_d = nc.dram_tensor("wo", [128, NP, C], BF16, kind="ExternalInput").ap()
    cos_d = nc.dram_tensor("cosT", [128, T], BF16, kind="ExternalInput").ap()
    sin_d = nc.dram_tensor("sinT", [128, T], BF16, kind="ExternalInput").ap()
    psw_d = nc.dram_tensor("psw", [128, 128], BF16, kind="ExternalInput").ap()
    tri_d = nc.dram_tensor("tri", [128, 128], BF16, kind="ExternalInput").ap()
    out_d = nc.dram_tensor("out", [T, C], BF16, kind="ExternalOutput").ap()

    with tile.TileContext(nc) as tc:
        with ExitStack() as ctx:
            pers = ctx.enter_context(tc.tile_pool(name="pers", bufs=1))
            wts = ctx.enter_context(tc.tile_pool(name="wts", bufs=1))
            xcp = ctx.enter_context(tc.tile_pool(name="xcp", bufs=2))
            wrk = ctx.enter_context(tc.tile_pool(name="wrk", bufs=3))
            rwk = ctx.enter_context(tc.tile_pool(name="rwk", bufs=2))
            ps_sc = ctx.enter_context(
                tc.tile_pool(name="ps_sc", bufs=2, space="PSUM")
            )
            ps_py = ctx.enter_context(
                tc.tile_pool(name="ps_py", bufs=2, space="PSUM")
            )

            # ---- persistent tensors
            qkT = {}
            for p in range(NP):
                for s in "qk":
                    qkT[(p, s)] = pers.tile([128, T], BF16, name=f"qkT_{p}_{s}")
            vext = pers.tile([128, TT, HL, 128], BF16)
            yT = [pers.tile([128, T], BF16, name=f"yT_{r}") for r in range(NP)]
            rrec = [pers.tile([128, 1024], F32, name=f"rrec{i}") for i in range(2)]

            nc.gpsimd.memset(vext[:, :, :, 0:1], 1.0)
            nc.gpsimd.memset(vext[:, :, :, 1:64], 0.0)

            # ---- weights / tables
            wq_sb = wts.tile([128, NP, KT, 128], BF16)
            wk_sb = wts.tile([128, NP, KT, 128], BF16)
            wv_sb = wts.tile([128, KT, 512], BF16)
            wo_sb = wts.tile([128, NP, C], BF16)
            cos_sb = wts.tile([128, T], BF16)
            sin_sb = wts.tile([128, T], BF16)
            psw_sb = wts.tile([128, 128], BF16)
            tri_sb = wts.tile([128, 128], BF16)
            xc = {}

            def load_xc(qc):
                xc[qc] = xcp.tile([128, KT, 512], BF16, tag="xc", name=f"xc{qc}")
                nc.sync.dma_start(
                    xc[qc][:], xT_d[:, :, qc * 512 : (qc + 1) * 512]
                )

            load_xc(0)
            nc.sync.dma_start(wv_sb[:], wv_d)
            nc.sync.dma_start(wq_sb[:], wq_d)
            nc.sync.dma_start(wk_sb[:], wk_d)
            nc.sync.dma_start(cos_sb[:], cos_d)
            nc.sync.dma_start(sin_sb[:], sin_d)
            nc.sync.dma_start(psw_sb[:], psw_d)
            load_xc(1)
            nc.sync.dma_start(tri_sb[:], tri_d)
            nc.sync.dma_start(wo_sb[:], wo_d)

            def vproj_tile(tt):
                # one 128-row tile of the value projection -> vext
                qc = tt // 4
                toff = (tt % 4) * 128
                ps = ps_sc.tile([128, 1024], F32, tag="sc", name="vps")
                for kt in range(KT):
                    nc.tensor.matmul(
                        ps[:, 0:512],
                        xc[qc][:, kt, toff : toff + 128],
                        wv_sb[:, kt],
                        start=(kt == 0),
                        stop=(kt == KT - 1),
                    )
                nc.vector.tensor_copy(
                    vext[:, tt, :, 64:128],
                    ps[:, 0:512].rearrange("p (h d) -> p h d", h=HL),
                )

            def qkproj_half(p, qc, half, ps=None):
                # half 0: q into cols 0:512; half 1: k into cols 512:1024
                if ps is None:
                    ps = ps_py.tile(
                        [128, 1024], F32, tag="py", name=f"qk_{p}_{qc}"
                    )
                w_sb = wq_sb if half == 0 else wk_sb
                sl = slice(half * 512, half * 512 + 512)
                for kt in range(KT):
                    nc.tensor.matmul(
                        ps[:, sl], w_sb[:, p, kt], xc[qc][:, kt],
                        start=(kt == 0), stop=(kt == KT - 1),
                    )
                return ps

            def rope(p, qc, ps):
                # qkT[.] = cos*proj + psw_perm(sin*proj), per q/k half of ps
                lo, hi = qc * 512, (qc + 1) * 512
                qsb = rwk.tile([128, 1024], BF16, tag="qsb")
                nc.vector.tensor_copy(qsb[:], ps[:])
                u = rwk.tile([128, 1024], BF16, tag="u")
                qcs = rwk.tile([128, 1024], BF16, tag="qcs")
                for h in range(2):
                    sl = slice(h * 512, h * 512 + 512)
                    nc.vector.tensor_tensor(
                        u[:, sl], qsb[:, sl], sin_sb[:, lo:hi], MULT
                    )
                    nc.vector.tensor_tensor(
                        qcs[:, sl], qsb[:, sl], cos_sb[:, lo:hi], MULT
                    )
                # rotation matmuls overwrite ps (proj value already in qsb)
                for h in range(2):
                    sl = slice(h * 512, h * 512 + 512)
                    nc.tensor.matmul(
                        ps[:, sl], psw_sb[:], u[:, sl], start=True, stop=True
                    )
                for h, s in ((0, "q"), (1, "k")):
                    sl = slice(h * 512, h * 512 + 512)
                    nc.vector.tensor_tensor(
                        qkT[(p, s)][:, lo:hi], ps[:, sl], qcs[:, sl], ADD
                    )

            def outproj_mt(qc, mt_i, pool_tag="sc"):
                # one 128-query-row slab: all 1024 out cols in one psum tile
                mt = qc * 4 + mt_i
                mlo, mhi = mt * 128, (mt + 1) * 128
                if pool_tag == "sc":
                    ps = ps_sc.tile([128, 1024], F32, tag="sc", name="ops")
                else:
                    ps = ps_py.tile([128, 1024], F32, tag="py", name="opy")
                for cc in range(2):
                    sl = slice(cc * 512, (cc + 1) * 512)
                    for r in range(NP):
                        nc.tensor.matmul(
                            ps[:, sl], yT[r][:, mlo:mhi], wo_sb[:, r, sl],
                            start=(r == 0), stop=(r == NP - 1),
                        )
                ob = wrk.tile([128, 1024], BF16, tag="ob")
                nc.vector.tensor_copy(ob[:], ps[:])
                nc.sync.dma_start(out_d[mlo:mhi, :], ob[:])

            def att_stream(p, qc, fillers=()):
                # fillers: list of (after_kt, closure) emitted into the PE
                # stream right after that tile's AV matmuls
                qT = qkT[(p, "q")]
                kT = qkT[(p, "k")]
                lo, hi = qc * 512, (qc + 1) * 512
                nkt = (qc + 1) * 4
                fillers = sorted(fillers, key=lambda f: f[0])
                psy = ps_py.tile([128, 1024], F32, tag="py", name=f"psy_{p}_{qc}")

                tiles = []  # (kt, tr, sc_tile, a_tile)

                def emit_scores(kt):
                    klo, khi = kt * 128, (kt + 1) * 128
                    off = klo - lo
                    tr = off if off > 0 else 0
                    st = ps_sc.tile([128, 1024], F32, tag="sc", name="st")
                    nc.tensor.matmul(
                        st[:, tr:512],
                        kT[0:64, klo:khi], qT[0:64, lo + tr : hi],
                        start=True, stop=True,
                    )
                    nc.tensor.matmul(
                        st[:, 512 + tr : 1024],
                        kT[64:128, klo:khi], qT[64:128, lo + tr : hi],
                        start=True, stop=True,
                    )
                    a = wrk.tile([128, 1024], BF16, tag="a", name="a")
                    s3 = st[:].rearrange("p (h n) -> p h n", h=2)
                    a3 = a[:].rearrange("p (h n) -> p h n", h=2)
                    nc.scalar.activation(
                        a3[:, :, tr:512], s3[:, :, tr:512], EXPF, scale=SCALE
                    )
                    if off >= 0:
                        for h in range(2):
                            nc.vector.tensor_tensor(
                                a3[:, h, off : off + 128],
                                a3[:, h, off : off + 128],
                                tri_sb[:],
                                MULT,
                            )
                    tiles.append((kt, tr, st, a))

                def emit_av(i):
                    kt, tr, st, a = tiles[i]
                    first, last = kt == 0, kt == nkt - 1
                    for h in range(2):
                        nc.tensor.matmul(
                            psy[0:128, h * 512 + tr : h * 512 + 512],
                            vext[:, kt, 2 * p + h, 0:128],
                            a[:, h * 512 + tr : h * 512 + 512],
                            start=first, stop=last,
                        )

                emit_scores(0)
                for kt in range(1, nkt):
                    emit_scores(kt)
                    emit_av(kt - 1)
                    while fillers and fillers[0][0] <= kt - 1:
                        fillers.pop(0)[1]()
                emit_av(nkt - 1)
                for _, f in fillers:
                    f()

                # denominator reciprocal. The av stationary operand is
                # [ones | 63 pad | v], so psy row 0 is the denominator (the
                # only partition GpSimd partition_broadcast can source on HW)
                # and the y data sits at partitions 64:128 (legal DVE base).
                rr = rrec[p % 2]
                for h in range(2):
                    sl = slice(h * 512, h * 512 + 512)
                    nc.vector.reciprocal_approx_fast(rr[0:1, sl], psy[0:1, sl])
                return psy

            def finish_normalize(p, qc, psy):
                lo, hi = qc * 512, (qc + 1) * 512
                rr = rrec[p % 2]
                bcs = wrk.tile([128, 1024], F32, tag="bcs")
                for h in range(2):
                    sl = slice(h * 512, h * 512 + 512)
                    nc.gpsimd.partition_broadcast(bcs[0:128, sl], rr[0:1, sl])
                tba = wrk.tile([128, 512], BF16, tag="tba")
                nc.vector.tensor_tensor(
                    tba[64:128, :], psy[64:128, 0:512], bcs[64:128, 0:512], MULT
                )
                nc.vector.tensor_tensor(
                    yT[p][64:128, lo:hi],
                    psy[64:128, 512:1024], bcs[64:128, 512:1024], MULT,
                )
                nc.sync.dma_start(yT[p][0:64, lo:hi], tba[64:128, :])

            # ---- emission ------------------------------------------------
            for tt in range(4):
                vproj_tile(tt)
            for p in range(NP):
                ps = qkproj_half(p, 0, 0)
                qkproj_half(p, 0, 1, ps)
                rope(p, 0, ps)

            for qc in range(QC):
                if qc + 2 < QC:
                    load_xc(qc + 2)
                for p in range(NP):
                    ctx_ps = {}

                    def fill_q(p=p, qc=qc):
                        ctx_ps["ps"] = qkproj_half(p, qc + 1, 0)

                    def fill_k(p=p, qc=qc):
                        qkproj_half(p, qc + 1, 1, ctx_ps["ps"])

                    nkt = (qc + 1) * 4
                    if qc == QC - 1:
                        # last round: no projections left; feed the PE with
                        # output-projection slabs on the py tag mid-stream
                        fillers = [
                            (5, lambda mt_i=p: outproj_mt(qc - 1, mt_i, "py")),
                        ]
                    elif qc > 0:
                        fillers = [
                            (nkt - 6, fill_q),
                            (nkt - 3, fill_k),
                        ]
                    else:
                        fillers = []
                    psy = att_stream(p, qc, fillers)
                    if qc == 0 and qc < QC - 1:
                        ps = qkproj_half(p, qc + 1, 0)
                        qkproj_half(p, qc + 1, 1, ps)
                        ctx_ps["ps"] = ps
                    if qc < QC - 1 and p in (1, 2):
                        base = (qc + 1) * 4
                        for tt in (base + 2 * (p - 1), base + 2 * (p - 1) + 1):
                            vproj_tile(tt)
                    if 0 < qc < QC - 1:
                        outproj_mt(qc - 1, p)
                    if qc < QC - 1:
                        rope(p, qc + 1, ctx_ps["ps"])
                    finish_normalize(p, qc, psy)
            for mt_i in range(4):
                outproj_mt(QC - 1, mt_i)

    nc.compile()
    return nc


def _host_tables():
    half = D // 2
    freq = np.exp(-math.log(10000.0) * np.arange(half) / half).astype(np.float64)
    ang = np.arange(T, dtype=np.float64)[None, :] * freq[:, None]  # [32, T]
    cos32 = np.cos(ang).astype(np.float32)
    sin32 = np.sin(ang).astype(np.float32)
    cosT = np.tile(cos32, (4, 1))                                   # [128, T]
    sinT = np.concatenate([sin32, -sin32, sin32, -sin32], axis=0)   # [128, T]
    psw = np.zeros((128, 128), np.float32)
    psw[np.arange(128) ^ 32, np.arange(128)] = 1.0
    kk = np.arange(128)[:, None]
    qq = np.arange(128)[None, :]
    tri = (qq >= kk).astype(np.float32)
    return cosT, sinT, psw, tri


def _bf(a):
    return np.ascontiguousarray(a.astype(ml_dtypes.bfloat16))


def _pack_weights(w_qkv, w_out, hg):
    lo, hi = hg * HL, (hg + 1) * HL
    wqf = w_qkv[:, 0:C].reshape(C, H, D)[:, lo:hi]       # [C, 8, D]
    wkf = w_qkv[:, C : 2 * C].reshape(C, H, D)[:, lo:hi]
    wvf = w_qkv[:, 2 * C : 3 * C].reshape(C, H, D)[:, lo:hi]

    def pack_qk(w):
        a = w.reshape(KT, 128, NP, 2, D)
        return _bf(a.transpose(1, 2, 0, 3, 4).reshape(128, NP, KT, 128))

    wq = pack_qk(wqf)
    wk = pack_qk(wkf)
    wv = _bf(wvf.reshape(KT, 128, HL * D).transpose(1, 0, 2))
    wo_l = w_out.reshape(H, D, C)[lo:hi].reshape(NP, 128, C)
    wo = _bf(wo_l.transpose(1, 0, 2))
    return wq, wk, wv, wo


def _prepare_in_maps(x, w_qkv, w_out):
    x = np.asarray(x, dtype=np.float32)
    w_qkv = np.asarray(w_qkv, dtype=np.float32)
    w_out = np.asarray(w_out, dtype=np.float32)

    cosT, sinT, psw, tri = _host_tables()
    cosT, sinT, psw, tri = map(_bf, (cosT, sinT, psw, tri))
    packs = [_pack_weights(w_qkv, w_out, hg) for hg in range(2)]
    xTs = [
        _bf(x[b].T.reshape(KT, 128, T).transpose(1, 0, 2)) for b in range(B)
    ]

    in_maps = []
    for c in range(8):
        b, hg = c // 2, c % 2
        wq, wk, wv, wo = packs[hg]
        in_maps.append(
            {
                "xT": xTs[b], "wq": wq, "wk": wk, "wv": wv, "wo": wo,
                "cosT": cosT, "sinT": sinT, "psw": psw,
                "tri": tri,
            }
        )
    return in_maps


def kernel(x, w_qkv, w_out):
    if "nc" not in _CACHE:
        _CACHE["nc"] = _build_nc()
    nc = _CACHE["nc"]

    in_maps = _prepare_in_maps(x, w_qkv, w_out)
    res = run_bass_kernel_spmd(nc, in_maps, core_ids=list(range(8)))
    outs = [res.results[c]["out"].astype(np.float32) for c in range(8)]
    y = np.stack([outs[2 * b] + outs[2 * b + 1] for b in range(B)], axis=0)
    return y.astype(np.float32)


# revision 15
# speedup vs baseline: 1.0160x; 1.0160x over previous
"""Causal self-attention (RoPE) Trainium2 Bass kernel, v2 (bf16).

Sharding: 8 cores = 4 batches x 2 head-groups. Core c handles batch c//2 and
heads (c%2)*8 .. (c%2)*8+7. Each core computes its QKV projection slice, RoPE,
causal flash-style attention in transposed layout, and a partial output
projection; the host sums the two partial projections per batch.

v2 changes vs baseline:
- all matmuls bf16 (fp32r ran in fp32-HIGH mode at half PE rate and kept the
  HAM clock gate cold for ~2/3 of the kernel)
- attention inner loop software-pipelined: scores for tile k+1 issue on PE
  before the AV matmuls of tile k, so exp (ACT) and mask (DVE) overlap PE
- reciprocal_approx_fast for softmax denominators (plain RECIPROCAL was 3.3us
  per call, 107us total)
- causal mask multiply only on the 128-wide diagonal triangle
- single pass over x feeds both the V and QK projections
- projection / attention / output-projection emission interleaved per
  512-query chunk so the PE queue always has ready work behind stalled ops
"""

import math
import numpy as np
from contextlib import ExitStack

import ml_dtypes

import concourse.bass as bass
import concourse.tile as tile
from concourse import bacc, mybir
from concourse.bass_utils import run_bass_kernel_spmd

F32 = mybir.dt.float32
BF16 = mybir.dt.bfloat16
EXPF = mybir.ActivationFunctionType.Exp
MULT = mybir.AluOpType.mult
ADD = mybir.AluOpType.add

B, T, C, H, D = 4, 2048, 1024, 16, 64
HL = 8            # local heads per core
NP = HL // 2      # head pairs per core
KT = C // 128     # contraction tiles for projections
TT = T // 128     # 128-row tiles of T
QC = T // 512     # 512-col chunks of T
SCALE = 1.0 / math.sqrt(D)

_CACHE = {}


def _build_nc():
    nc = bacc.Bacc("TRN2", debug=False, num_devices=8)

    xT_d = nc.dram_tensor("xT", [128, KT, T], BF16, kind="ExternalInput").ap()
    wq_d = nc.dram_tensor("wq", [128, NP, KT, 128], BF16, kind="ExternalInput").ap()
    wk_d = nc.dram_tensor("wk", [128, NP, KT, 128], BF16, kind="ExternalInput").ap()
    wv_d = nc.dram_tensor("wv", [128, KT, 512], BF16, kind="ExternalInput").ap()
    wo_d = nc.dram_tensor("wo", [128, NP, C], BF16, kind="ExternalInput").ap()
    cos_d = nc.dram_tensor("cosT", [128, T], BF16, kind="ExternalInput").ap()
    sin_d = nc.dram_tensor("sinT", [128, T], BF16, kind="ExternalInput").ap()
    psw_d = nc.dram_tensor("psw", [128, 128], BF16, kind="ExternalInput").ap()
    tri_d = nc.dram_tensor("tri", [128, 128], BF16, kind="ExternalInput").ap()
    out_d = nc.dram_tensor("out", [T, C], BF16, kind="ExternalOutput").ap()

    with tile.TileContext(nc) as tc:
        with ExitStack() as ctx:
            pers = ctx.enter_context(tc.tile_pool(name="pers", bufs=1))
            wts = ctx.enter_context(tc.tile_pool(name="wts", bufs=1))
            xcp = ctx.enter_context(tc.tile_pool(name="xcp", bufs=2))
            wrk = ctx.enter_context(tc.tile_pool(name="wrk", bufs=3))
            rwk = ctx.enter_context(tc.tile_pool(name="rwk", bufs=2))
            ps_sc = ctx.enter_context(
                tc.tile_pool(name="ps_sc", bufs=2, space="PSUM")
            )
            ps_py = ctx.enter_context(
                tc.tile_pool(name="ps_py", bufs=2, space="PSUM")
            )

            # ---- persistent tensors
            qkT = {}
            for p in range(NP):
                for s in "qk":
                    qkT[(p, s)] = pers.tile([128, T], BF16, name=f"qkT_{p}_{s}")
            vext = pers.tile([128, TT, HL, 128], BF16)
            yT = [pers.tile([128, T], BF16, name=f"yT_{r}") for r in range(NP)]
            rrec = [pers.tile([128, 1024], F32, name=f"rrec{i}") for i in range(2)]

            nc.gpsimd.memset(vext[:, :, :, 0:1], 1.0)
            nc.gpsimd.memset(vext[:, :, :, 1:64], 0.0)

            # ---- weights / tables
            wq_sb = wts.tile([128, NP, KT, 128], BF16)
            wk_sb = wts.tile([128, NP, KT, 128], BF16)
            wv_sb = wts.tile([128, KT, 512], BF16)
            wo_sb = wts.tile([128, NP, C], BF16)
            cos_sb = wts.tile([128, T], BF16)
            sin_sb = wts.tile([128, T], BF16)
            psw_sb = wts.tile([128, 128], BF16)
            tri_sb = wts.tile([128, 128], BF16)
            xc = {}

            def load_xc(qc, split=False):
                xc[qc] = xcp.tile([128, KT, 512], BF16, tag="xc", name=f"xc{qc}")
                lo = qc * 512
                if split:
                    nc.sync.dma_start(
                        xc[qc][:, :, 0:256], xT_d[:, :, lo : lo + 256]
                    )
                    nc.sync.dma_start(
                        xc[qc][:, :, 256:512], xT_d[:, :, lo + 256 : lo + 512]
                    )
                else:
                    nc.sync.dma_start(xc[qc][:], xT_d[:, :, lo : lo + 512])

            load_xc(0, split=True)
            nc.sync.dma_start(wv_sb[:], wv_d)
            nc.sync.dma_start(wq_sb[:], wq_d)
            nc.sync.dma_start(wk_sb[:], wk_d)
            nc.sync.dma_start(cos_sb[:], cos_d)
            nc.sync.dma_start(sin_sb[:], sin_d)
            nc.sync.dma_start(psw_sb[:], psw_d)
            load_xc(1)
            nc.sync.dma_start(tri_sb[:], tri_d)
            nc.sync.dma_start(wo_sb[:], wo_d)

            def vproj_tile(tt):
                # one 128-row tile of the value projection -> vext
                qc = tt // 4
                toff = (tt % 4) * 128
                ps = ps_sc.tile([128, 1024], F32, tag="sc", name="vps")
                for kt in range(KT):
                    nc.tensor.matmul(
                        ps[:, 0:512],
                        xc[qc][:, kt, toff : toff + 128],
                        wv_sb[:, kt],
                        start=(kt == 0),
                        stop=(kt == KT - 1),
                    )
                nc.vector.tensor_copy(
                    vext[:, tt, :, 64:128],
                    ps[:, 0:512].rearrange("p (h d) -> p h d", h=HL),
                )

            def qkproj_half(p, qc, half, ps=None):
                # half 0: q into cols 0:512; half 1: k into cols 512:1024
                if ps is None:
                    ps = ps_py.tile(
                        [128, 1024], F32, tag="py", name=f"qk_{p}_{qc}"
                    )
                w_sb = wq_sb if half == 0 else wk_sb
                sl = slice(half * 512, half * 512 + 512)
                for kt in range(KT):
                    nc.tensor.matmul(
                        ps[:, sl], w_sb[:, p, kt], xc[qc][:, kt],
                        start=(kt == 0), stop=(kt == KT - 1),
                    )
                return ps

            def rope(p, qc, ps):
                # qkT[.] = cos*proj + psw_perm(sin*proj), per q/k half of ps
                lo, hi = qc * 512, (qc + 1) * 512
                qsb = rwk.tile([128, 1024], BF16, tag="qsb")
                nc.vector.tensor_copy(qsb[:], ps[:])
                u = rwk.tile([128, 1024], BF16, tag="u")
                qcs = rwk.tile([128, 1024], BF16, tag="qcs")
                for h in range(2):
                    sl = slice(h * 512, h * 512 + 512)
                    nc.vector.tensor_tensor(
                        u[:, sl], qsb[:, sl], sin_sb[:, lo:hi], MULT
                    )
                    nc.vector.tensor_tensor(
                        qcs[:, sl], qsb[:, sl], cos_sb[:, lo:hi], MULT
                    )
                # rotation matmuls overwrite ps (proj value already in qsb)
                for h in range(2):
                    sl = slice(h * 512, h * 512 + 512)
                    nc.tensor.matmul(
                        ps[:, sl], psw_sb[:], u[:, sl], start=True, stop=True
                    )
                for h, s in ((0, "q"), (1, "k")):
                    sl = slice(h * 512, h * 512 + 512)
                    nc.vector.tensor_tensor(
                        qkT[(p, s)][:, lo:hi], ps[:, sl], qcs[:, sl], ADD
                    )

            def outproj_mt(qc, mt_i, pool_tag="sc"):
                # one 128-query-row slab: all 1024 out cols in one psum tile
                mt = qc * 4 + mt_i
                mlo, mhi = mt * 128, (mt + 1) * 128
                if pool_tag == "sc":
                    ps = ps_sc.tile([128, 1024], F32, tag="sc", name="ops")
                else:
                    ps = ps_py.tile([128, 1024], F32, tag="py", name="opy")
                for cc in range(2):
                    sl = slice(cc * 512, (cc + 1) * 512)
                    for r in range(NP):
                        nc.tensor.matmul(
                            ps[:, sl], yT[r][:, mlo:mhi], wo_sb[:, r, sl],
                            start=(r == 0), stop=(r == NP - 1),
                        )
                ob = wrk.tile([128, 1024], BF16, tag="ob")
                nc.vector.tensor_copy(ob[:], ps[:])
                nc.sync.dma_start(out_d[mlo:mhi, :], ob[:])

            def att_stream(p, qc, fillers=()):
                # fillers: list of (after_kt, closure) emitted into the PE
                # stream right after that tile's AV matmuls
                qT = qkT[(p, "q")]
                kT = qkT[(p, "k")]
                lo, hi = qc * 512, (qc + 1) * 512
                nkt = (qc + 1) * 4
                fillers = sorted(fillers, key=lambda f: f[0])
                psy = ps_py.tile([128, 1024], F32, tag="py", name=f"psy_{p}_{qc}")

                tiles = []  # (kt, tr, sc_tile, a_tile)

                def emit_scores(kt):
                    klo, khi = kt * 128, (kt + 1) * 128
                    off = klo - lo
                    tr = off if off > 0 else 0
                    st = ps_sc.tile([128, 1024], F32, tag="sc", name="st")
                    nc.tensor.matmul(
                        st[:, tr:512],
                        kT[0:64, klo:khi], qT[0:64, lo + tr : hi],
                        start=True, stop=True,
                    )
                    nc.tensor.matmul(
                        st[:, 512 + tr : 1024],
                        kT[64:128, klo:khi], qT[64:128, lo + tr : hi],
                        start=True, stop=True,
                    )
                    a = wrk.tile([128, 1024], BF16, tag="a", name="a")
                    s3 = st[:].rearrange("p (h n) -> p h n", h=2)
                    a3 = a[:].rearrange("p (h n) -> p h n", h=2)
                    nc.scalar.activation(
                        a3[:, :, tr:512], s3[:, :, tr:512], EXPF, scale=SCALE
                    )
                    if off >= 0:
                        for h in range(2):
                            nc.vector.tensor_tensor(
                                a3[:, h, off : off + 128],
                                a3[:, h, off : off + 128],
                                tri_sb[:],
                                MULT,
                            )
                    tiles.append((kt, tr, st, a))

                def emit_av(i):
                    kt, tr, st, a = tiles[i]
                    first, last = kt == 0, kt == nkt - 1
                    for h in range(2):
                        nc.tensor.matmul(
                            psy[0:128, h * 512 + tr : h * 512 + 512],
                            vext[:, kt, 2 * p + h, 0:128],
                            a[:, h * 512 + tr : h * 512 + 512],
                            start=first, stop=last,
                        )

                emit_scores(0)
                for kt in range(1, nkt):
                    emit_scores(kt)
                    emit_av(kt - 1)
                    while fillers and fillers[0][0] <= kt - 1:
                        fillers.pop(0)[1]()
                emit_av(nkt - 1)
                for _, f in fillers:
                    f()

                # denominator reciprocal. The av stationary operand is
                # [ones | 63 pad | v], so psy row 0 is the denominator (the
                # only partition GpSimd partition_broadcast can source on HW)
                # and the y data sits at partitions 64:128 (legal DVE base).
                rr = rrec[p % 2]
                for h in range(2):
                    sl = slice(h * 512, h * 512 + 512)
                    nc.vector.reciprocal_approx_fast(rr[0:1, sl], psy[0:1, sl])
                return psy

            def finish_normalize(p, qc, psy):
                lo, hi = qc * 512, (qc + 1) * 512
                rr = rrec[p % 2]
                bcs = wrk.tile([128, 1024], F32, tag="bcs")
                for h in range(2):
                    sl = slice(h * 512, h * 512 + 512)
                    nc.gpsimd.partition_broadcast(bcs[0:128, sl], rr[0:1, sl])
                tba = wrk.tile([128, 512], BF16, tag="tba")
                nc.vector.tensor_tensor(
                    tba[64:128, :], psy[64:128, 0:512], bcs[64:128, 0:512], MULT
                )
                nc.vector.tensor_tensor(
                    yT[p][64:128, lo:hi],
                    psy[64:128, 512:1024], bcs[64:128, 512:1024], MULT,
                )
                nc.sync.dma_start(yT[p][0:64, lo:hi], tba[64:128, :])

            # ---- emission ------------------------------------------------
            for tt in range(4):
                vproj_tile(tt)
            for p in range(NP):
                ps = qkproj_half(p, 0, 0)
                qkproj_half(p, 0, 1, ps)
                rope(p, 0, ps)

            for qc in range(QC):
                if qc + 2 < QC:
                    load_xc(qc + 2)
                for p in range(NP):
                    psy = att_stream(p, qc)
                    if qc < QC - 1:
                        ps = qkproj_half(p, qc + 1, 0)
                        qkproj_half(p, qc + 1, 1, ps)
                        rope(p, qc + 1, ps)
                        if p in (1, 2):
                            base = (qc + 1) * 4
                            for tt in (base + 2 * (p - 1), base + 2 * (p - 1) + 1):
                                vproj_tile(tt)
                    if qc > 0:
                        outproj_mt(qc - 1, p)
                    finish_normalize(p, qc, psy)
            for mt_i in range(4):
                outproj_mt(QC - 1, mt_i)

    nc.compile()
    return nc


def _host_tables():
    half = D // 2
    freq = np.exp(-math.log(10000.0) * np.arange(half) / half).astype(np.float64)
    ang = np.arange(T, dtype=np.float64)[None, :] * freq[:, None]  # [32, T]
    cos32 = np.cos(ang).astype(np.float32)
    sin32 = np.sin(ang).astype(np.float32)
    cosT = np.tile(cos32, (4, 1))                                   # [128, T]
    sinT = np.concatenate([sin32, -sin32, sin32, -sin32], axis=0)   # [128, T]
    psw = np.zeros((128, 128), np.float32)
    psw[np.arange(128) ^ 32, np.arange(128)] = 1.0
    kk = np.arange(128)[:, None]
    qq = np.arange(128)[None, :]
    tri = (qq >= kk).astype(np.float32)
    return cosT, sinT, psw, tri


def _bf(a):
    return np.ascontiguousarray(a.astype(ml_dtypes.bfloat16))


def _pack_weights(w_qkv, w_out, hg):
    lo, hi = hg * HL, (hg + 1) * HL
    wqf = w_qkv[:, 0:C].reshape(C, H, D)[:, lo:hi]       # [C, 8, D]
    wkf = w_qkv[:, C : 2 * C].reshape(C, H, D)[:, lo:hi]
    wvf = w_qkv[:, 2 * C : 3 * C].reshape(C, H, D)[:, lo:hi]

    def pack_qk(w):
        a = w.reshape(KT, 128, NP, 2, D)
        return _bf(a.transpose(1, 2, 0, 3, 4).reshape(128, NP, KT, 128))

    wq = pack_qk(wqf)
    wk = pack_qk(wkf)
    wv = _bf(wvf.reshape(KT, 128, HL * D).transpose(1, 0, 2))
    wo_l = w_out.reshape(H, D, C)[lo:hi].reshape(NP, 128, C)
    wo = _bf(wo_l.transpose(1, 0, 2))
    return wq, wk, wv, wo


def _prepare_in_maps(x, w_qkv, w_out):
    x = np.asarray(x, dtype=np.float32)
    w_qkv = np.asarray(w_qkv, dtype=np.float32)
    w_out = np.asarray(w_out, dtype=np.float32)

    cosT, sinT, psw, tri = _host_tables()
    cosT, sinT, psw, tri = map(_bf, (cosT, sinT, psw, tri))
    packs = [_pack_weights(w_qkv, w_out, hg) for hg in range(2)]
    xTs = [
        _bf(x[b].T.reshape(KT, 128, T).transpose(1, 0, 2)) for b in range(B)
    ]

    in_maps = []
    for c in range(8):
        b, hg = c // 2, c % 2
        wq, wk, wv, wo = packs[hg]
        in_maps.append(
            {
                "xT": xTs[b], "wq": wq, "wk": wk, "wv": wv, "wo": wo,
                "cosT": cosT, "sinT": sinT, "psw": psw,
                "tri": tri,
            }
        )
    return in_maps


def kernel(x, w_qkv, w_out):
    if "nc" not in _CACHE:
        _CACHE["nc"] = _build_nc()
    nc = _CACHE["nc"]

    in_maps = _prepare_in_maps(x, w_qkv, w_out)
    res = run_bass_kernel_spmd(nc, in_maps, core_ids=list(range(8)))
    outs = [res.results[c]["out"].astype(np.float32) for c in range(8)]
    y = np.stack([outs[2 * b] + outs[2 * b + 1] for b in range(B)], axis=0)
    return y.astype(np.float32)
